# revision 5
# baseline (speedup 1.0000x reference)
"""Trainium2 Bass kernel for nn_MultiHeadAttention_88888643158578.

Math (see reference): single shared attention head (HS=64) over [B=4, T=2048,
E=1024]; the NH=16 identical head outputs concatenated then projected by Wp is
equivalent to head @ Wp_eff where Wp_eff = sum of the 16 row-blocks of Wp.
Softmax max-subtraction is skipped (logits are O(1)); the softmax denominator
is carried as an extra "ones" column in V and divided out after the final
projection (all linear, so exactly equivalent).

Sharding: core c -> batch b=c//2, query half h=c%2 in "zigzag" superblocks of
512 rows (h=0: abs spans {0,3}, h=1: {1,2}) to balance causal work. Keys are
PERMUTED per core (host-side) to local order [own-span-alpha, own-span-beta,
restA, restB] so that the causal structure is core-invariant in local
coordinates: static triangle masks on the block-diagonal, a static skip of the
above-diagonal rectangle, and two data-driven (input bias vector) rest-slots.
Each core computes k/v for all 2048 keys and q for its 1024 rows from the same
transposed input xT (host-transposed, bf16), attention entirely on-chip, then
out = (head_unnorm @ Wp_eff) * (1/d) with d from an augmented matmul column.
Bias bp is added on the host (exact, f32).

Schedule (v2): DMAs issued in consumption order (wqkv, xT span chunks, wp,
vm); on-chip-only warm-up matmuls at the head of the PE queue ramp the HAM
duty cycle during the input-DMA window; attention is emitted as flat
key-block units (scores -> exp -> PV) with independent projection work
interleaved at the exp-wait points; the output projection for the first query
span is interleaved into the second attention phase so its compute and store
DMA overlap attention instead of trailing it.
"""

import numpy as np
import ml_dtypes
from contextlib import ExitStack

import concourse.bass as bass
import concourse.tile as tile
from concourse import bacc, mybir
from concourse.bass_utils import run_bass_kernel_spmd

BF16 = ml_dtypes.bfloat16

B, T, E, HS = 4, 2048, 1024, 64
NH = E // HS
SB = 512          # superblock (query span / key superblock)
KB = 128          # key block
NQ = 1024         # queries per core
NET = E // 128    # e-tiles
WARM_N = 12       # HAM-ramp warm-up matmuls (512 cols each)

F32 = mybir.dt.float32
BF = mybir.dt.bfloat16

_CACHE = {}


def build_program():
    nc = bacc.Bacc("TRN2", target_bir_lowering=False, debug=False)

    xT = nc.dram_tensor("xT", [E, T], BF, kind="ExternalInput").ap()
    # host pre-tiled to the SBUF layout: [128, NET*3*HS], contiguous rows
    wqkv = nc.dram_tensor("wqkv", [128, NET * 3 * HS], BF,
                          kind="ExternalInput").ap()
    wp = nc.dram_tensor("wp", [HS + 1, E + 1], BF, kind="ExternalInput").ap()
    vm = nc.dram_tensor("vm", [128, 2], F32, kind="ExternalInput").ap()
    out = nc.dram_tensor("out", [NQ, E], BF, kind="ExternalOutput").ap()

    with tile.TileContext(nc) as tc:
        with ExitStack() as ctx:
            consts = ctx.enter_context(tc.tile_pool(name="consts", bufs=1))
            sb = ctx.enter_context(tc.tile_pool(name="sb", bufs=1))
            ps = ctx.enter_context(tc.tile_pool(name="ps", bufs=1, space="PSUM"))

            # warm-up source: on-chip only, ready as soon as vector runs
            warm = consts.tile([128, 512], BF, name="warm")
            nc.vector.memset(warm[:], 0.0)

            # ---- DMAs in consumption order ----
            w3_sb = consts.tile([128, NET, 3 * HS], BF, name="w3_sb")
            nc.sync.dma_start(w3_sb[:], wqkv.rearrange("p (a h) -> p a h",
                                                       h=3 * HS))
            wq_sb = w3_sb[:, :, 0:HS]
            wk_sb = w3_sb[:, :, HS:2 * HS]
            wv_sb = w3_sb[:, :, 2 * HS:3 * HS]

            xT_sb = consts.tile([128, NET, T], BF, name="xT_sb")
            xTr = xT.rearrange("(a p) t -> p a t", p=128)
            for g in (0, 1, 2, 3):
                nc.sync.dma_start(xT_sb[:, :, g * SB:(g + 1) * SB],
                                  xTr[:, :, g * SB:(g + 1) * SB])

            wp_sb = consts.tile([HS + 1, E + 1], BF, name="wp_sb")
            nc.sync.dma_start(wp_sb[:], wp[:])
            vm_sb = consts.tile([128, 2], F32, name="vm_sb")
            nc.sync.dma_start(vm_sb[:], vm[:])

            # identity for PE transpose, at both partition halves
            ident = consts.tile([128, 64], BF, name="ident")
            nc.gpsimd.memset(ident[0:64, :], 0.0)
            nc.gpsimd.affine_select(
                out=ident[0:64, :], in_=ident[0:64, :],
                compare_op=mybir.AluOpType.not_equal, fill=1.0,
                base=0, pattern=[[-1, 64]], channel_multiplier=1,
            )
            nc.gpsimd.dma_start(ident[64:128, :], ident[0:64, :])
            # canonical 128x128 causal triangle: tri[ki, qi] = 1 iff qi >= ki
            tri = consts.tile([128, 128], BF, name="tri")
            nc.gpsimd.memset(tri[:], 1.0)
            nc.gpsimd.affine_select(
                out=tri[:], in_=tri[:],
                compare_op=mybir.AluOpType.is_ge, fill=0.0,
                base=0, pattern=[[1, 128]], channel_multiplier=-1,
            )

            # ---- persistent working tiles ----
            # kT2: [0:64] = key blocks 0..7, [64:128] = key blocks 8..15
            kT2 = sb.tile([128, 8, KB], BF, name="kT2")
            # qT2: qT duplicated on both partition halves
            qT2 = sb.tile([128, NQ], BF, name="qT2")
            # vT split by where the packed projection left it
            vT_b = sb.tile([128, NQ], BF, name="vT_b")   # rows 64:128, keys 0:1024
            vT_a = sb.tile([64, NQ], BF, name="vT_a")    # rows 0:64, keys 1024:2048
            v_sb = sb.tile([128, T // KB, HS + 1], BF, name="v_sb")
            nc.vector.memset(v_sb[:, :, HS:HS + 1], 1.0)
            vq0r = sb.tile([128, 4, HS + 1], BF, name="vq0r")
            vq1r = sb.tile([128, 4, HS + 1], BF, name="vq1r")
            headT_sb = sb.tile([HS + 1, NQ], BF, name="headT_sb")

            # ---- PE warm-up: on-chip-only matmuls ramp the HAM activity
            # monitor to full duty while the input DMA streams in ----
            for w in range(WARM_N):
                pw = ps.tile([128, 512], F32, name=f"warm_{w}", tag="p1",
                             bufs=2)
                nc.tensor.matmul(pw[:], lhsT=warm[:, 0:128], rhs=warm[:],
                                 start=True, stop=True)

            # ---- packed projections: per (span, e-tile) two col-tiled
            # matmuls (out partition halves 0/64) sharing the moving xT span.
            # spans 0,1: (k -> rows 0:64, v -> rows 64:128)
            # spans 2,3: (v -> rows 0:64, k -> rows 64:128)
            kv_state = {}

            def kv_open(ts):
                kv_state[ts] = ps.tile([128, SB], F32, name=f"pkv_{ts}",
                                       tag="p1", bufs=2)

            def kv_step(ts, et):
                pkv = kv_state[ts]
                lo_w, hi_w = (wk_sb, wv_sb) if ts < 2 else (wv_sb, wk_sb)
                nc.tensor.matmul(
                    pkv[0:64, :], lhsT=lo_w[:, et, :],
                    rhs=xT_sb[:, et, ts * SB:(ts + 1) * SB],
                    start=(et == 0), stop=(et == NET - 1),
                )
                nc.tensor.matmul(
                    pkv[64:128, :], lhsT=hi_w[:, et, :],
                    rhs=xT_sb[:, et, ts * SB:(ts + 1) * SB],
                    start=(et == 0), stop=(et == NET - 1),
                )

            def kv_close(ts):
                pkv = kv_state.pop(ts)
                if ts < 2:
                    for i in range(4):
                        nc.vector.tensor_copy(
                            kT2[0:64, 4 * ts + i, :],
                            pkv[0:64, i * KB:(i + 1) * KB])
                    nc.vector.tensor_copy(
                        vT_b[64:128, ts * SB:(ts + 1) * SB], pkv[64:128, :])
                else:
                    nc.vector.tensor_copy(
                        vT_a[:, (ts - 2) * SB:(ts - 1) * SB], pkv[0:64, :])
                    for i in range(4):
                        nc.vector.tensor_copy(
                            kT2[64:128, 4 * (ts - 2) + i, :],
                            pkv[64:128, i * KB:(i + 1) * KB])

            def emit_kv_span(ts):
                kv_open(ts)
                for et in range(NET):
                    kv_step(ts, et)
                kv_close(ts)

            def emit_q():
                pq = ps.tile([128, SB], F32, name="pq", tag="p1", bufs=2)
                for et in range(NET):
                    nc.tensor.matmul(
                        pq[0:64, :], lhsT=wq_sb[:, et, :],
                        rhs=xT_sb[:, et, 0:SB],
                        start=(et == 0), stop=(et == NET - 1),
                    )
                    nc.tensor.matmul(
                        pq[64:128, :], lhsT=wq_sb[:, et, :],
                        rhs=xT_sb[:, et, SB:2 * SB],
                        start=(et == 0), stop=(et == NET - 1),
                    )
                nc.vector.tensor_copy(qT2[0:64, 0:SB], pq[0:64, :])
                nc.vector.tensor_copy(qT2[64:128, SB:2 * SB], pq[64:128, :])
                nc.gpsimd.dma_start(qT2[64:128, 0:SB], qT2[0:64, 0:SB])
                nc.gpsimd.dma_start(qT2[0:64, SB:2 * SB], qT2[64:128, SB:2 * SB])

            def emit_transpose(kb):
                if kb in done_tr:
                    return
                done_tr.add(kb)
                if kb < 8:
                    tsrc = vT_b[64:128, kb * KB:(kb + 1) * KB]
                    idn = ident[64:128, :]
                else:
                    tsrc = vT_a[:, (kb - 8) * KB:(kb - 7) * KB]
                    idn = ident[0:64, :]
                pt = ps.tile([128, 64], BF, name=f"pt_{kb}", tag="tr", bufs=2)
                nc.tensor.transpose(pt[:], tsrc, idn)
                nc.vector.tensor_copy(v_sb[:, kb, 0:HS], pt[:])
                if 8 <= kb < 12:
                    nc.vector.tensor_scalar(
                        vq0r[:, kb - 8, 0:HS], pt[:], vm_sb[:, 0:1], None,
                        op0=mybir.AluOpType.mult)
                    nc.vector.tensor_copy(vq0r[:, kb - 8, HS:HS + 1],
                                          vm_sb[:, 0:1])
                elif kb >= 12:
                    nc.vector.tensor_scalar(
                        vq1r[:, kb - 12, 0:HS], pt[:], vm_sb[:, 1:2], None,
                        op0=mybir.AluOpType.mult)
                    nc.vector.tensor_copy(vq1r[:, kb - 12, HS:HS + 1],
                                          vm_sb[:, 1:2])
            done_tr = set()

            # ---- attention as flat units: one key block x one query span.
            # unit = (kb, krow, off, tri?, v_hi_sel). krow: 0 -> kT2[0:64],
            # 1 -> kT2[64:128]. Score -> exp -> (filler) -> PV accumulate.
            att_state = {}

            def att_open(qs, n_units):
                att_state[qs] = {
                    "pv": ps.tile([HS + 1, SB], F32, name=f"pv_{qs}",
                                  tag="pv", bufs=1),
                    "bi": 0, "n": n_units,
                }

            def att_unit(qs, kb, off, is_diag, v_sel, filler=None):
                st = att_state[qs]
                krow = kT2[0:64, kb, :] if kb < 8 else kT2[64:128, kb - 8, :]
                qrow = (qT2[0:64, qs * SB + off:(qs + 1) * SB] if kb < 8 else
                        qT2[64:128, qs * SB + off:(qs + 1) * SB])
                s2 = ps.tile([128, SB], F32, name=f"s2_{qs}_{kb}",
                             tag="s2", bufs=3)
                nc.tensor.matmul(s2[:, off:SB], lhsT=krow, rhs=qrow,
                                 start=True, stop=True)
                emit_transpose(kb)
                ex = sb.tile([128, SB], BF, name=f"ex_{qs}_{kb}",
                             tag="ex", bufs=6)
                nc.scalar.activation(
                    ex[:, off:SB], s2[:, off:SB],
                    mybir.ActivationFunctionType.Exp,
                )
                if is_diag:
                    nc.vector.tensor_mul(
                        ex[:, off:off + KB], ex[:, off:off + KB], tri[:])
                if filler is not None:
                    filler()
                if v_sel == "v":
                    v_t = v_sb[:, kb, :]
                elif v_sel == "vq0":
                    v_t = vq0r[:, kb - 8, :]
                else:
                    v_t = vq1r[:, kb - 12, :]
                nc.tensor.matmul(
                    st["pv"][:, off:SB], lhsT=v_t, rhs=ex[:, off:SB],
                    start=(st["bi"] == 0), stop=(st["bi"] == st["n"] - 1),
                )
                st["bi"] += 1

            def att_close(qs):
                st = att_state.pop(qs)
                assert st["bi"] == st["n"]
                nc.vector.tensor_copy(headT_sb[:, qs * SB:(qs + 1) * SB],
                                      st["pv"])

            # ---- output projection + normalization for one 128-row block ----
            def emit_outproj(tb, use_scalar=False):
                lhs = headT_sb[:, tb * 128:(tb + 1) * 128]
                d_ps = ps.tile([128, 1], F32, name=f"d_{tb}", tag="p1", bufs=2)
                nc.tensor.matmul(d_ps[:], lhsT=lhs, rhs=wp_sb[:, E:E + 1],
                                 start=True, stop=True)
                rd = sb.tile([128, 1], F32, name=f"rd_{tb}", tag="rd", bufs=2)
                nc.vector.reciprocal(rd[:], d_ps[:])
                ob = sb.tile([128, E], BF, name=f"ob_{tb}", tag="ob", bufs=3)
                for fs in range(E // SB):
                    o_ps = ps.tile([128, SB], F32, name=f"o_{tb}_{fs}",
                                   tag="p1", bufs=2)
                    nc.tensor.matmul(
                        o_ps[:], lhsT=lhs, rhs=wp_sb[:, fs * SB:(fs + 1) * SB],
                        start=True, stop=True,
                    )
                    if fs == 1 and use_scalar:
                        nc.scalar.mul(ob[:, fs * SB:(fs + 1) * SB], o_ps[:],
                                      rd[:, 0:1])
                    else:
                        nc.vector.tensor_scalar(
                            ob[:, fs * SB:(fs + 1) * SB], o_ps[:], rd[:, 0:1],
                            None, op0=mybir.AluOpType.mult,
                        )
                nc.sync.dma_start(out[tb * 128:(tb + 1) * 128, :], ob[:])

            # ================= emission schedule =================
            emit_kv_span(0)          # keys 0:512 (needs xT chunk 0)
            emit_q()                 # q both spans (needs chunks 0,1)

            # qs0 phase 1: diagonal blocks only (need kv span 0 + q)
            att_open(0, 8)
            for m in range(4):
                att_unit(0, m, 128 * m, True, "v")

            emit_kv_span(2)          # keys 1024:1536 (needs chunk 2)

            # qs0 phase 2: rest-slot blocks (bias-gated), kv span 1
            # interleaved at the exp-wait points
            kv_open(1)

            def f_kv1(lo):
                def f():
                    kv_step(1, 2 * lo)
                    kv_step(1, 2 * lo + 1)
                return f
            for m in range(4):
                att_unit(0, m + 8, 0, False, "vq0", filler=f_kv1(m))
            kv_close(1)
            att_close(0)

            # qs1: 16 units; kv span 3 fills the first 4 pairs, the output
            # projection for query span 0 fills the last 4 pairs
            att_open(1, 16)
            kv_open(3)

            def f_kv3(lo):
                def f():
                    kv_step(3, 2 * lo)
                    kv_step(3, 2 * lo + 1)
                return f

            for m in range(4):
                att_unit(1, m, 0, False, "v", filler=f_kv3(m))
                att_unit(1, m + 8, 0, False, "v")
            kv_close(3)
            for m in range(4, 8):
                att_unit(1, m, 128 * (m - 4), True, "v")
                att_unit(1, m + 8, 0, False, "vq1",
                         filler=(lambda tb=m - 4: emit_outproj(tb)))
            att_close(1)

            # output projection for query span 1
            for tb in range(4, NQ // 128):
                emit_outproj(tb, use_scalar=True)

    nc.compile()
    return nc


def _core_layout(h):
    if h == 0:
        alpha, beta, rest = 0, 3, [1, 2]
        vmask = np.array([0.0, 1.0], np.float32)   # (qs0-restA, qs1-restB)
    else:
        alpha, beta, rest = 1, 2, [0, 3]
        vmask = np.array([1.0, 0.0], np.float32)
    perm_sb = [alpha, beta] + rest
    key_perm = np.concatenate([np.arange(s * SB, (s + 1) * SB) for s in perm_sb])
    return alpha, beta, key_perm, vmask


def kernel(x, Wq, Wk, Wv, Wp, bp):
    x = np.asarray(x, np.float32)
    Wq = np.asarray(Wq, np.float32)
    Wk = np.asarray(Wk, np.float32)
    Wv = np.asarray(Wv, np.float32)
    Wp = np.asarray(Wp, np.float32)
    bp = np.asarray(bp, np.float32)

    if "nc" not in _CACHE:
        _CACHE["nc"] = build_program()
    nc = _CACHE["nc"]

    Wp_eff = Wp.reshape(NH, HS, E).sum(axis=0, dtype=np.float32)
    wp_aug = np.zeros((HS + 1, E + 1), np.float32)
    wp_aug[:HS, :E] = Wp_eff
    wp_aug[HS, E] = 1.0

    wqkv_full = np.concatenate([Wq / np.sqrt(HS), Wk, Wv], axis=1)
    # pre-tile to SBUF layout [128, NET*3*HS] so the DMA rows are contiguous
    wqkv_b = np.ascontiguousarray(
        wqkv_full.reshape(NET, 128, 3 * HS).transpose(1, 0, 2)
        .reshape(128, NET * 3 * HS)).astype(BF16)
    wp_b = wp_aug.astype(BF16)

    in_maps = []
    metas = []
    for c in range(8):
        b, h = c // 2, c % 2
        alpha, beta, key_perm, vmask = _core_layout(h)
        xT = np.ascontiguousarray(x[b].T[:, key_perm]).astype(BF16)
        in_maps.append({
            "xT": xT, "wqkv": wqkv_b, "wp": wp_b,
            "vm": np.broadcast_to(vmask, (128, 2)).copy(),
        })
        metas.append((b, alpha, beta))

    trace = bool(_CACHE.get("trace"))
    if trace:
        import axon_prof
        axon_prof.install()
    try:
        res = run_bass_kernel_spmd(
            nc, in_maps, core_ids=list(range(8)),
            trace=trace, trace_cores=[0] if trace else None,
        )
    except Exception:
        # transient NRT device errors have been observed; retry once
        res = run_bass_kernel_spmd(
            nc, in_maps, core_ids=list(range(8)),
            trace=trace, trace_cores=[0] if trace else None,
        )
    _CACHE["last_exec_time_ns"] = res.exec_time_ns
    _CACHE["last_results"] = res

    out_full = np.empty((B, T, E), np.float32)
    for c in range(8):
        b, alpha, beta = metas[c]
        o = res.results[c]["out"].astype(np.float32)
        out_full[b, alpha * SB:(alpha + 1) * SB] = o[:SB]
        out_full[b, beta * SB:(beta + 1) * SB] = o[SB:]
    out_full += bp[None, None, :]
    return out_full


# revision 6
# speedup vs baseline: 1.1350x; 1.1350x over previous
"""Trainium2 Bass kernel for nn_MultiHeadAttention_88888643158578.

Math (see reference): single shared attention head (HS=64) over [B=4, T=2048,
E=1024]; the NH=16 identical head outputs concatenated then projected by Wp is
equivalent to head @ Wp_eff where Wp_eff = sum of the 16 row-blocks of Wp.
Softmax max-subtraction is skipped (logits are O(1)); the softmax denominator
is carried as an extra "ones" column in V and divided out after the final
projection (all linear, so exactly equivalent).

Sharding: core c -> batch b=c//2, query half h=c%2 in "zigzag" superblocks of
512 rows (h=0: abs spans {0,3}, h=1: {1,2}) to balance causal work. Keys are
PERMUTED per core (host-side) to local order [own-span-alpha, own-span-beta,
restA, restB] so that the causal structure is core-invariant in local
coordinates: static triangle masks on the block-diagonal, a static skip of the
above-diagonal rectangle, and two data-driven (input bias vector) rest-slots.
Each core computes k/v for all 2048 keys and q for its 1024 rows from the same
transposed input xT (host-transposed, bf16), attention entirely on-chip, then
out = (head_unnorm @ Wp_eff) * (1/d) with d from an augmented matmul column.
Bias bp is added on the host (exact, f32).

Schedule (v2): DMAs issued in consumption order (wqkv, xT span chunks, wp,
vm); on-chip-only warm-up matmuls at the head of the PE queue ramp the HAM
duty cycle during the input-DMA window; attention is emitted as flat
key-block units (scores -> exp -> PV) with independent projection work
interleaved at the exp-wait points; the output projection for the first query
span is interleaved into the second attention phase so its compute and store
DMA overlap attention instead of trailing it.
"""

import numpy as np
import ml_dtypes
from contextlib import ExitStack

import concourse.bass as bass
import concourse.tile as tile
from concourse import bacc, mybir
from concourse.bass_utils import run_bass_kernel_spmd

BF16 = ml_dtypes.bfloat16

B, T, E, HS = 4, 2048, 1024, 64
NH = E // HS
SB = 512          # superblock (query span / key superblock)
KB = 128          # key block
NQ = 1024         # queries per core
NET = E // 128    # e-tiles
WARM_N = 0        # HAM-ramp warm-up matmuls (512 cols each); 0 = disabled
                  # (warm-ups drain the HAM full-duty budget -> net loss)

F32 = mybir.dt.float32
BF = mybir.dt.bfloat16

_CACHE = {}


def build_program():
    nc = bacc.Bacc("TRN2", target_bir_lowering=False, debug=False)

    xT = nc.dram_tensor("xT", [E, T], BF, kind="ExternalInput").ap()
    # host pre-tiled to the SBUF layout: [128, NET*3*HS], contiguous rows
    wqkv = nc.dram_tensor("wqkv", [128, NET * 3 * HS], BF,
                          kind="ExternalInput").ap()
    wp = nc.dram_tensor("wp", [HS + 1, E + 1], BF, kind="ExternalInput").ap()
    vm = nc.dram_tensor("vm", [128, 2], F32, kind="ExternalInput").ap()
    out = nc.dram_tensor("out", [NQ, E], BF, kind="ExternalOutput").ap()

    with tile.TileContext(nc) as tc:
        with ExitStack() as ctx:
            consts = ctx.enter_context(tc.tile_pool(name="consts", bufs=1))
            sb = ctx.enter_context(tc.tile_pool(name="sb", bufs=1))
            ps = ctx.enter_context(tc.tile_pool(name="ps", bufs=1, space="PSUM"))

            # warm-up source: on-chip only, ready as soon as vector runs
            warm = consts.tile([128, 512], BF, name="warm")
            nc.vector.memset(warm[:], 0.0)

            # ---- DMAs in consumption order ----
            w3_sb = consts.tile([128, NET, 3 * HS], BF, name="w3_sb")
            nc.sync.dma_start(w3_sb[:], wqkv.rearrange("p (a h) -> p a h",
                                                       h=3 * HS))
            wq_sb = w3_sb[:, :, 0:HS]
            wk_sb = w3_sb[:, :, HS:2 * HS]
            wv_sb = w3_sb[:, :, 2 * HS:3 * HS]

            xT_sb = consts.tile([128, NET, T], BF, name="xT_sb")
            xTr = xT.rearrange("(a p) t -> p a t", p=128)
            for g in (0, 1, 2, 3):
                nc.sync.dma_start(xT_sb[:, :, g * SB:(g + 1) * SB],
                                  xTr[:, :, g * SB:(g + 1) * SB])

            wp_sb = consts.tile([HS + 1, E + 1], BF, name="wp_sb")
            nc.sync.dma_start(wp_sb[:], wp[:])
            vm_sb = consts.tile([128, 2], F32, name="vm_sb")
            nc.sync.dma_start(vm_sb[:], vm[:])

            # identity for PE transpose, at both partition halves
            ident = consts.tile([128, 64], BF, name="ident")
            nc.gpsimd.memset(ident[0:64, :], 0.0)
            nc.gpsimd.affine_select(
                out=ident[0:64, :], in_=ident[0:64, :],
                compare_op=mybir.AluOpType.not_equal, fill=1.0,
                base=0, pattern=[[-1, 64]], channel_multiplier=1,
            )
            nc.gpsimd.dma_start(ident[64:128, :], ident[0:64, :])
            # canonical 128x128 causal triangle: tri[ki, qi] = 1 iff qi >= ki
            tri = consts.tile([128, 128], BF, name="tri")
            nc.gpsimd.memset(tri[:], 1.0)
            nc.gpsimd.affine_select(
                out=tri[:], in_=tri[:],
                compare_op=mybir.AluOpType.is_ge, fill=0.0,
                base=0, pattern=[[1, 128]], channel_multiplier=-1,
            )

            # ---- persistent working tiles ----
            # kT2: [0:64] = key blocks 0..7, [64:128] = key blocks 8..15
            kT2 = sb.tile([128, 8, KB], BF, name="kT2")
            # qT2: qT duplicated on both partition halves
            qT2 = sb.tile([128, NQ], BF, name="qT2")
            # vT split by where the packed projection left it
            vT_b = sb.tile([128, NQ], BF, name="vT_b")   # rows 64:128, keys 0:1024
            vT_a = sb.tile([64, NQ], BF, name="vT_a")    # rows 0:64, keys 1024:2048
            v_sb = sb.tile([128, T // KB, HS + 1], BF, name="v_sb")
            nc.vector.memset(v_sb[:, :, HS:HS + 1], 1.0)
            vq0r = sb.tile([128, 4, HS + 1], BF, name="vq0r")
            vq1r = sb.tile([128, 4, HS + 1], BF, name="vq1r")
            headT_sb = sb.tile([HS + 1, NQ], BF, name="headT_sb")

            # ---- PE warm-up: on-chip-only matmuls ramp the HAM activity
            # monitor to full duty while the input DMA streams in ----
            for w in range(WARM_N):
                pw = ps.tile([128, 512], F32, name=f"warm_{w}", tag="p1",
                             bufs=2)
                nc.tensor.matmul(pw[:], lhsT=warm[:, 0:128], rhs=warm[:],
                                 start=True, stop=True)

            # ---- packed projections: per (span, e-tile) two col-tiled
            # matmuls (out partition halves 0/64) sharing the moving xT span.
            # spans 0,1: (k -> rows 0:64, v -> rows 64:128)
            # spans 2,3: (v -> rows 0:64, k -> rows 64:128)
            kv_state = {}

            def kv_open(ts):
                kv_state[ts] = ps.tile([128, SB], F32, name=f"pkv_{ts}",
                                       tag="p1", bufs=2)

            def kv_step(ts, et):
                pkv = kv_state[ts]
                lo_w, hi_w = (wk_sb, wv_sb) if ts < 2 else (wv_sb, wk_sb)
                nc.tensor.matmul(
                    pkv[0:64, :], lhsT=lo_w[:, et, :],
                    rhs=xT_sb[:, et, ts * SB:(ts + 1) * SB],
                    start=(et == 0), stop=(et == NET - 1),
                )
                nc.tensor.matmul(
                    pkv[64:128, :], lhsT=hi_w[:, et, :],
                    rhs=xT_sb[:, et, ts * SB:(ts + 1) * SB],
                    start=(et == 0), stop=(et == NET - 1),
                )

            def kv_close(ts):
                pkv = kv_state.pop(ts)
                if ts < 2:
                    for i in range(4):
                        nc.vector.tensor_copy(
                            kT2[0:64, 4 * ts + i, :],
                            pkv[0:64, i * KB:(i + 1) * KB])
                    nc.vector.tensor_copy(
                        vT_b[64:128, ts * SB:(ts + 1) * SB], pkv[64:128, :])
                else:
                    nc.vector.tensor_copy(
                        vT_a[:, (ts - 2) * SB:(ts - 1) * SB], pkv[0:64, :])
                    for i in range(4):
                        nc.vector.tensor_copy(
                            kT2[64:128, 4 * (ts - 2) + i, :],
                            pkv[64:128, i * KB:(i + 1) * KB])

            def emit_kv_span(ts):
                kv_open(ts)
                for et in range(NET):
                    kv_step(ts, et)
                kv_close(ts)

            def emit_q():
                pq = ps.tile([128, SB], F32, name="pq", tag="p1", bufs=2)
                for et in range(NET):
                    nc.tensor.matmul(
                        pq[0:64, :], lhsT=wq_sb[:, et, :],
                        rhs=xT_sb[:, et, 0:SB],
                        start=(et == 0), stop=(et == NET - 1),
                    )
                    nc.tensor.matmul(
                        pq[64:128, :], lhsT=wq_sb[:, et, :],
                        rhs=xT_sb[:, et, SB:2 * SB],
                        start=(et == 0), stop=(et == NET - 1),
                    )
                nc.vector.tensor_copy(qT2[0:64, 0:SB], pq[0:64, :])
                nc.vector.tensor_copy(qT2[64:128, SB:2 * SB], pq[64:128, :])
                nc.gpsimd.dma_start(qT2[64:128, 0:SB], qT2[0:64, 0:SB])
                nc.gpsimd.dma_start(qT2[0:64, SB:2 * SB], qT2[64:128, SB:2 * SB])

            def emit_transpose(kb):
                if kb in done_tr:
                    return
                done_tr.add(kb)
                if kb < 8:
                    tsrc = vT_b[64:128, kb * KB:(kb + 1) * KB]
                    idn = ident[64:128, :]
                else:
                    tsrc = vT_a[:, (kb - 8) * KB:(kb - 7) * KB]
                    idn = ident[0:64, :]
                pt = ps.tile([128, 64], BF, name=f"pt_{kb}", tag="tr", bufs=2)
                nc.tensor.transpose(pt[:], tsrc, idn)
                nc.vector.tensor_copy(v_sb[:, kb, 0:HS], pt[:])
                if 8 <= kb < 12:
                    nc.vector.tensor_scalar(
                        vq0r[:, kb - 8, 0:HS], pt[:], vm_sb[:, 0:1], None,
                        op0=mybir.AluOpType.mult)
                    nc.vector.tensor_copy(vq0r[:, kb - 8, HS:HS + 1],
                                          vm_sb[:, 0:1])
                elif kb >= 12:
                    nc.vector.tensor_scalar(
                        vq1r[:, kb - 12, 0:HS], pt[:], vm_sb[:, 1:2], None,
                        op0=mybir.AluOpType.mult)
                    nc.vector.tensor_copy(vq1r[:, kb - 12, HS:HS + 1],
                                          vm_sb[:, 1:2])
            done_tr = set()

            # ---- attention as flat units: one key block x one query span.
            # unit = (kb, krow, off, tri?, v_hi_sel). krow: 0 -> kT2[0:64],
            # 1 -> kT2[64:128]. Score -> exp -> (filler) -> PV accumulate.
            att_state = {}

            def att_open(qs, n_units):
                att_state[qs] = {
                    "pv": ps.tile([HS + 1, SB], F32, name=f"pv_{qs}",
                                  tag="pv", bufs=1),
                    "bi": 0, "n": n_units,
                }

            def att_unit(qs, kb, off, is_diag, v_sel, filler=None):
                st = att_state[qs]
                krow = kT2[0:64, kb, :] if kb < 8 else kT2[64:128, kb - 8, :]
                qrow = (qT2[0:64, qs * SB + off:(qs + 1) * SB] if kb < 8 else
                        qT2[64:128, qs * SB + off:(qs + 1) * SB])
                s2 = ps.tile([128, SB], F32, name=f"s2_{qs}_{kb}",
                             tag="s2", bufs=3)
                nc.tensor.matmul(s2[:, off:SB], lhsT=krow, rhs=qrow,
                                 start=True, stop=True)
                emit_transpose(kb)
                ex = sb.tile([128, SB], BF, name=f"ex_{qs}_{kb}",
                             tag="ex", bufs=6)
                nc.scalar.activation(
                    ex[:, off:SB], s2[:, off:SB],
                    mybir.ActivationFunctionType.Exp,
                )
                if is_diag:
                    nc.vector.tensor_mul(
                        ex[:, off:off + KB], ex[:, off:off + KB], tri[:])
                if filler is not None:
                    filler()
                if v_sel == "v":
                    v_t = v_sb[:, kb, :]
                elif v_sel == "vq0":
                    v_t = vq0r[:, kb - 8, :]
                else:
                    v_t = vq1r[:, kb - 12, :]
                nc.tensor.matmul(
                    st["pv"][:, off:SB], lhsT=v_t, rhs=ex[:, off:SB],
                    start=(st["bi"] == 0), stop=(st["bi"] == st["n"] - 1),
                )
                st["bi"] += 1

            def att_close(qs):
                st = att_state.pop(qs)
                assert st["bi"] == st["n"]
                nc.vector.tensor_copy(headT_sb[:, qs * SB:(qs + 1) * SB],
                                      st["pv"])

            # ---- output projection + normalization for one 128-row block ----
            def emit_outproj(tb, use_scalar=False):
                lhs = headT_sb[:, tb * 128:(tb + 1) * 128]
                d_ps = ps.tile([128, 1], F32, name=f"d_{tb}", tag="p1", bufs=2)
                nc.tensor.matmul(d_ps[:], lhsT=lhs, rhs=wp_sb[:, E:E + 1],
                                 start=True, stop=True)
                rd = sb.tile([128, 1], F32, name=f"rd_{tb}", tag="rd", bufs=2)
                nc.vector.reciprocal(rd[:], d_ps[:])
                ob = sb.tile([128, E], BF, name=f"ob_{tb}", tag="ob", bufs=3)
                for fs in range(E // SB):
                    o_ps = ps.tile([128, SB], F32, name=f"o_{tb}_{fs}",
                                   tag="p1", bufs=2)
                    nc.tensor.matmul(
                        o_ps[:], lhsT=lhs, rhs=wp_sb[:, fs * SB:(fs + 1) * SB],
                        start=True, stop=True,
                    )
                    if fs == 1 and use_scalar:
                        nc.scalar.mul(ob[:, fs * SB:(fs + 1) * SB], o_ps[:],
                                      rd[:, 0:1])
                    else:
                        nc.vector.tensor_scalar(
                            ob[:, fs * SB:(fs + 1) * SB], o_ps[:], rd[:, 0:1],
                            None, op0=mybir.AluOpType.mult,
                        )
                nc.sync.dma_start(out[tb * 128:(tb + 1) * 128, :], ob[:])

            # ================= emission schedule =================
            emit_kv_span(0)          # keys 0:512 (needs xT chunk 0)
            emit_q()                 # q both spans (needs chunks 0,1)

            # qs0 phase 1: diagonal blocks only (need kv span 0 + q)
            att_open(0, 8)
            for m in range(4):
                att_unit(0, m, 128 * m, True, "v")

            emit_kv_span(2)          # keys 1024:1536 (needs chunk 2)

            # qs0 phase 2: rest-slot blocks (bias-gated), kv span 1
            # interleaved at the exp-wait points
            kv_open(1)

            def f_kv1(lo):
                def f():
                    kv_step(1, 2 * lo)
                    kv_step(1, 2 * lo + 1)
                return f
            for m in range(4):
                att_unit(0, m + 8, 0, False, "vq0", filler=f_kv1(m))
            kv_close(1)
            att_close(0)

            # qs1: 16 units; kv span 3 fills the first 4 pairs, the output
            # projection for query span 0 fills the last 4 pairs
            att_open(1, 16)
            kv_open(3)

            def f_kv3(lo):
                def f():
                    kv_step(3, 2 * lo)
                    kv_step(3, 2 * lo + 1)
                return f

            for m in range(4):
                att_unit(1, m, 0, False, "v", filler=f_kv3(m))
                att_unit(1, m + 8, 0, False, "v")
            kv_close(3)
            for m in range(4, 8):
                att_unit(1, m, 128 * (m - 4), True, "v")
                att_unit(1, m + 8, 0, False, "vq1",
                         filler=(lambda tb=m - 4: emit_outproj(tb)))
            att_close(1)

            # output projection for query span 1
            for tb in range(4, NQ // 128):
                emit_outproj(tb, use_scalar=True)

    nc.compile()
    return nc


def _core_layout(h):
    if h == 0:
        alpha, beta, rest = 0, 3, [1, 2]
        vmask = np.array([0.0, 1.0], np.float32)   # (qs0-restA, qs1-restB)
    else:
        alpha, beta, rest = 1, 2, [0, 3]
        vmask = np.array([1.0, 0.0], np.float32)
    perm_sb = [alpha, beta] + rest
    key_perm = np.concatenate([np.arange(s * SB, (s + 1) * SB) for s in perm_sb])
    return alpha, beta, key_perm, vmask


def kernel(x, Wq, Wk, Wv, Wp, bp):
    x = np.asarray(x, np.float32)
    Wq = np.asarray(Wq, np.float32)
    Wk = np.asarray(Wk, np.float32)
    Wv = np.asarray(Wv, np.float32)
    Wp = np.asarray(Wp, np.float32)
    bp = np.asarray(bp, np.float32)

    if "nc" not in _CACHE:
        _CACHE["nc"] = build_program()
    nc = _CACHE["nc"]

    Wp_eff = Wp.reshape(NH, HS, E).sum(axis=0, dtype=np.float32)
    wp_aug = np.zeros((HS + 1, E + 1), np.float32)
    wp_aug[:HS, :E] = Wp_eff
    wp_aug[HS, E] = 1.0

    wqkv_full = np.concatenate([Wq / np.sqrt(HS), Wk, Wv], axis=1)
    # pre-tile to SBUF layout [128, NET*3*HS] so the DMA rows are contiguous
    wqkv_b = np.ascontiguousarray(
        wqkv_full.reshape(NET, 128, 3 * HS).transpose(1, 0, 2)
        .reshape(128, NET * 3 * HS)).astype(BF16)
    wp_b = wp_aug.astype(BF16)

    in_maps = []
    metas = []
    for c in range(8):
        b, h = c // 2, c % 2
        alpha, beta, key_perm, vmask = _core_layout(h)
        xT = np.ascontiguousarray(x[b].T[:, key_perm]).astype(BF16)
        in_maps.append({
            "xT": xT, "wqkv": wqkv_b, "wp": wp_b,
            "vm": np.broadcast_to(vmask, (128, 2)).copy(),
        })
        metas.append((b, alpha, beta))

    trace = bool(_CACHE.get("trace"))
    if trace:
        import axon_prof
        axon_prof.install()
    try:
        res = run_bass_kernel_spmd(
            nc, in_maps, core_ids=list(range(8)),
            trace=trace, trace_cores=[0] if trace else None,
        )
    except Exception:
        # transient NRT device errors have been observed; retry once
        res = run_bass_kernel_spmd(
            nc, in_maps, core_ids=list(range(8)),
            trace=trace, trace_cores=[0] if trace else None,
        )
    _CACHE["last_exec_time_ns"] = res.exec_time_ns
    _CACHE["last_results"] = res

    out_full = np.empty((B, T, E), np.float32)
    for c in range(8):
        b, alpha, beta = metas[c]
        o = res.results[c]["out"].astype(np.float32)
        out_full[b, alpha * SB:(alpha + 1) * SB] = o[:SB]
        out_full[b, beta * SB:(beta + 1) * SB] = o[SB:]
    out_full += bp[None, None, :]
    return out_full
